# revision 1
# baseline (speedup 1.0000x reference)
"""Trainium2 Bass kernel for nn_EvaluatorNetwork.

Network (per sample):
  sep = per-column spectral decomposition of image  -> (128, 128, 128)
  x = concat([sep, mask_embedding]) -> (134, 128, 128)
  conv0 4x4 s2 (134->256) + b0, lrelu          -> (256, 64, 64)
  conv1 4x4 s2 (256->512), inorm, lrelu        -> (512, 32, 32)
  conv2 4x4 s2 (512->1024), inorm, lrelu       -> (1024, 16, 16)
  conv3 4x4 s2 (1024->1024), inorm, lrelu      -> (1024, 8, 8)
  avgpool -> (1024,); head 1024->128 + b4      -> (128,)

Sharding: pure data parallel, batch 8 over 8 NeuronCores; weights replicated.

Math notes:
  sep[i,h,w] = colRT[i,h]*cos(2pi*i*w/W) + colJT[i,h]*sin(2pi*i*w/W)
    colRT = C @ img^T, colJT = S @ img^T,  C/S[w',i] = cos/sin(2pi*i*w'/W)/W
  b1..b3 cancel exactly through instance norm (constant channel shift), so
  they are ignored; b0 and b4 are applied.
  lrelu(y) = 0.2*y + 0.8*relu(y); relu(0.8*y) = 0.8*relu(y) (used so the
  per-partition-scale Relu activation can produce the relu part directly).
"""
from contextlib import ExitStack

import numpy as np

import concourse.bass as bass
import concourse.tile as tile
from concourse import bacc, mybir
from concourse.masks import make_identity

F32 = mybir.dt.float32
F16 = mybir.dt.float16

B, H, W = 8, 128, 128
EPS = 1e-5

# conv output spatial sizes
S0, S1, S2, S3 = 64, 32, 16, 8


def _build_nc():
    nc = bacc.Bacc("TRN2", target_bir_lowering=False, debug=False)

    # ---------------- DRAM parameters (per-core) ----------------
    d_img = nc.dram_tensor("img", [H, W], F32, kind="ExternalInput")
    d_maskim = nc.dram_tensor("maskim", [96, S0, S0], F16, kind="ExternalInput")
    d_C = nc.dram_tensor("twC", [W, W], F32, kind="ExternalInput")
    d_S = nc.dram_tensor("twS", [W, W], F32, kind="ExternalInput")
    d_c2 = nc.dram_tensor("twc2", [W, W], F16, kind="ExternalInput")
    d_s2 = nc.dram_tensor("tws2", [W, W], F16, kind="ExternalInput")
    d_w0s = nc.dram_tensor("w0s", [2, 16, 128, 128], F16, kind="ExternalInput")
    d_wm = nc.dram_tensor("wm", [96, 256], F16, kind="ExternalInput")
    d_w1 = nc.dram_tensor("w1l", [4, 2, 128, 16, 128], F16, kind="ExternalInput")
    d_w2 = nc.dram_tensor("w2l", [8, 4, 128, 16, 128], F16, kind="ExternalInput")
    d_w3 = nc.dram_tensor("w3l", [8, 8, 128, 16, 128], F16, kind="ExternalInput")
    d_w4 = nc.dram_tensor("w4l", [8, 128, 128], F16, kind="ExternalInput")
    d_b0 = nc.dram_tensor("b0t", [128, 4], F32, kind="ExternalInput")  # [b0, 0.8*b0]
    d_b4 = nc.dram_tensor("b4t", [128, 1], F32, kind="ExternalInput")
    d_out = nc.dram_tensor("out", [128], F32, kind="ExternalOutput")

    from contextlib import contextmanager

    @contextmanager
    def low_priority(tc, bump):
        orig = tc.cur_priority
        tc.cur_priority = orig + bump
        try:
            yield
        finally:
            tc.cur_priority = orig

    with tile.TileContext(nc) as tc, ExitStack() as ctx:
        const = ctx.enter_context(tc.tile_pool(name="const", bufs=1))
        act = ctx.enter_context(tc.tile_pool(name="act", bufs=1))
        wch = ctx.enter_context(tc.tile_pool(name="wch", bufs=21))
        ps = ctx.enter_context(tc.tile_pool(name="ps", bufs=3, space="PSUM"))
        tmp = ctx.enter_context(tc.tile_pool(name="tmp", bufs=6))
        tsp = ctx.enter_context(tc.tile_pool(name="tsp", bufs=3))

        # ---------------- constants / inputs ----------------
        ident = const.tile([128, 128], F32)
        make_identity(nc, ident[:])
        img32 = const.tile([128, 128], F32)
        nc.sync.dma_start(img32[:], d_img.ap())
        twC = const.tile([128, 128], F32)
        nc.sync.dma_start(twC[:], d_C.ap())
        twS = const.tile([128, 128], F32)
        nc.sync.dma_start(twS[:], d_S.ap())
        c2ated = const.tile([128, 128], F16)
        nc.sync.dma_start(c2ated[:], d_c2.ap())
        s2ated = const.tile([128, 128], F16)
        nc.sync.dma_start(s2ated[:], d_s2.ap())
        b0t = const.tile([128, 4], F32)
        nc.sync.dma_start(b0t[:], d_b0.ap())
        b4t = const.tile([128, 1], F32)
        nc.sync.dma_start(b4t[:], d_b4.ap())

        # mask im2col (host-computed): rows (kh,kw,ci), cols (oh,ow)
        mask_im = act.tile([96, S0, S0], F16)
        nc.sync.dma_start(mask_im[:], d_maskim.ap())

        # small weights resident
        w0s_sb = const.tile([128, 2, 16, 128], F16)
        src = d_w0s.ap().rearrange("m t k c -> k (m t) c")
        nc.sync.dma_start(w0s_sb[:].rearrange("k m t c -> k (m t) c"), src)
        wm_sb = const.tile([96, 256], F16)
        nc.sync.dma_start(wm_sb[:], d_wm.ap())
        w4_sb = const.tile([128, 8, 128], F16)
        nc.sync.dma_start(w4_sb[:], d_w4.ap().rearrange("t k c -> k t c"))

        # ---------------- spectral map ----------------
        pT = ps.tile([128, 128], F32, tag="ps")
        nc.tensor.transpose(pT[:], img32[:], ident[:])
        imgT32 = const.tile([128, 128], F32)
        nc.vector.tensor_copy(imgT32[:], pT[:])

        pR = ps.tile([128, 128], F32, tag="ps")
        nc.tensor.matmul(pR[:], twC[:], imgT32[:], start=True, stop=True)
        colRT = const.tile([128, 128], F16)
        nc.vector.tensor_copy(colRT[:], pR[:])
        pJ = ps.tile([128, 128], F32, tag="ps")
        nc.tensor.matmul(pJ[:], twS[:], imgT32[:], start=True, stop=True)
        colJT = const.tile([128, 128], F16)
        nc.vector.tensor_copy(colJT[:], pJ[:])

        sep_pad = act.tile([128, H + 2, W + 2], F16)
        # zero only the border strips (interior fully written below)
        nc.gpsimd.memset(sep_pad[:, 0, :], 0.0)
        nc.gpsimd.memset(sep_pad[:, H + 1, :], 0.0)
        nc.gpsimd.memset(sep_pad[:, :, 0], 0.0)
        nc.gpsimd.memset(sep_pad[:, :, W + 1], 0.0)

        h_chunks = [4, 4, 8] + [16] * 7  # small first chunks: conv0 starts sooner
        h0 = 0
        for HC in h_chunks:
            # A-term: colRT[i,h] bcast over w;  B-term: c2[i,w] bcast over h
            cR = colRT[:, h0:h0 + HC]
            aR = bass.AP(tensor=cR.tensor, offset=cR.offset,
                         ap=[cR.ap[0], [1, HC], [0, W]])
            cJ = colJT[:, h0:h0 + HC]
            aJ = bass.AP(tensor=cJ.tensor, offset=cJ.offset,
                         ap=[cJ.ap[0], [1, HC], [0, W]])
            c2a = c2ated[:, :]
            b2 = bass.AP(tensor=c2a.tensor, offset=c2a.offset,
                         ap=[c2a.ap[0], [0, HC], [1, W]])
            s2a = s2ated[:, :]
            b3 = bass.AP(tensor=s2a.tensor, offset=s2a.offset,
                         ap=[s2a.ap[0], [0, HC], [1, W]])
            t1 = tsp.tile([128, 16, W], F16, tag="tsp", name="t1")[:, :HC, :]
            nc.gpsimd.tensor_tensor(out=t1[:], in0=aR, in1=b2, op=mybir.AluOpType.mult)
            t2 = tsp.tile([128, 16, W], F16, tag="tsp", name="t2")[:, :HC, :]
            nc.vector.tensor_tensor(out=t2[:], in0=aJ, in1=b3, op=mybir.AluOpType.mult)
            nc.vector.tensor_tensor(out=sep_pad[:, 1 + h0:1 + h0 + HC, 1:1 + W],
                                    in0=t1[:], in1=t2[:], op=mybir.AluOpType.add)
            h0 += HC

        # ---------------- conv0: 134 -> 256, 128x128 -> 64x64 ----------------
        conv0_pad = [act.tile([128, S0 + 2, S0 + 2], F16, tag=f"c0p{m}", name=f"c0p{m}") for m in range(2)]
        with low_priority(tc, 400):
            for m in range(2):
                nc.gpsimd.memset(conv0_pad[m][:, 0, :], 0.0)
                nc.gpsimd.memset(conv0_pad[m][:, S0 + 1, :], 0.0)
                nc.gpsimd.memset(conv0_pad[m][:, :, 0], 0.0)
                nc.gpsimd.memset(conv0_pad[m][:, :, S0 + 1], 0.0)

        OHB0 = 8  # oh rows per chunk -> N = 8*64 = 512
        for m in range(2):
            for ch in range(S0 // OHB0):
                oh0 = ch * OHB0
                p0 = ps.tile([128, OHB0, S0], F32, tag="ps")
                for t in range(16):
                    kh, kw = t // 4, t % 4
                    rhs = sep_pad[:, kh + 2 * oh0: kh + 2 * oh0 + 2 * OHB0 - 1: 2,
                                  kw: kw + 2 * S0 - 1: 2]
                    nc.tensor.matmul(p0[:], w0s_sb[:, m, t, :], rhs,
                                     start=(t == 0), stop=False)
                nc.tensor.matmul(p0[:], wm_sb[:, m * 128:(m + 1) * 128],
                                 mask_im[:, oh0:oh0 + OHB0, :],
                                 start=False, stop=True)
                # evac: lrelu(x + b0) = 0.2*(x+b0) + 0.8*relu(x+b0)
                relu_t = tmp.tile([128, OHB0, S0], F16, tag="ev")
                nc.scalar.activation(out=relu_t[:], in_=p0[:],
                                     func=mybir.ActivationFunctionType.Relu,
                                     bias=b0t[:, 2 + m:3 + m], scale=0.8)
                lin_t = tmp.tile([128, OHB0, S0], F16, tag="ev")
                nc.vector.tensor_scalar(out=lin_t[:], in0=p0[:],
                                        scalar1=b0t[:, m:m + 1], scalar2=0.2,
                                        op0=mybir.AluOpType.add,
                                        op1=mybir.AluOpType.mult)
                nc.vector.tensor_tensor(
                    out=conv0_pad[m][:, 1 + oh0:1 + oh0 + OHB0, 1:1 + S0],
                    in0=lin_t[:], in1=relu_t[:], op=mybir.AluOpType.add)

        # ---------------- generic strided conv layer with inorm ----------------
        def conv_norm(x_pads, w_dram, nm, nk, osz, out_pads, pooled=None,
                      wk_provider=None):
            """x_pads: list of nk input padded tiles (128, isz+2, isz+2) fp16
            w_dram: DRAM (nm, nk, 128, 16, 128) fp16
            out_pads: list of nm output padded tiles, or None with pooled tile."""
            n_spatial = osz * osz
            # chunk rows so N <= 512
            ohb = max(1, min(osz, 512 // osz))
            nch = osz // ohb
            for m in range(nm):
                pm = ps.tile([128, osz, osz], F32, tag="ps")
                if wk_provider is not None:
                    wk = [wk_provider(m, k) for k in range(nk)]
                else:
                    wk = []
                    for k in range(nk):
                        wt = wch.tile([128, 16, 128], F16, tag="wch")
                        nc.sync.dma_start(wt[:], w_dram.ap()[m, k])
                        wk.append(wt)
                for ch in range(nch):
                    oh0 = ch * ohb
                    pslice = pm[:, oh0:oh0 + ohb, :]
                    first = True
                    for k in range(nk):
                        for t in range(16):
                            kh, kw = t // 4, t % 4
                            rhs = x_pads[k][:, kh + 2 * oh0: kh + 2 * oh0 + 2 * ohb - 1: 2,
                                            kw: kw + 2 * osz - 1: 2]
                            nc.tensor.matmul(pslice, wk[k][:, t, :], rhs,
                                             start=first,
                                             stop=(k == nk - 1 and t == 15))
                            first = False
                # instance norm stats over full spatial
                nsub = max(1, n_spatial // 512)
                sub = n_spatial // nsub
                stats = tmp.tile([128, nsub, 6], F32, tag="st")
                pf = pm[:].rearrange("p a b -> p (a b)")
                for s in range(nsub):
                    nc.vector.bn_stats(out=stats[:, s, :], in_=pf[:, s * sub:(s + 1) * sub])
                mv = tmp.tile([128, 2], F32, tag="mv")
                nc.vector.bn_aggr(out=mv[:], in_=stats[:])
                eps_t = tmp.tile([128, 1], F32, tag="eps")
                nc.vector.memset(eps_t[:], EPS)
                rs = tmp.tile([128, 1], F32, tag="rs")
                nc.scalar.activation(out=rs[:], in_=mv[:, 1:2],
                                     func=mybir.ActivationFunctionType.Sqrt,
                                     bias=eps_t[:], scale=1.0)
                nc.vector.reciprocal(out=rs[:], in_=rs[:])
                rs08 = tmp.tile([128, 1], F32, tag="rs08")
                nc.vector.tensor_scalar_mul(out=rs08[:], in0=rs[:], scalar1=0.8)
                rs02 = tmp.tile([128, 1], F32, tag="rs02")
                nc.vector.tensor_scalar_mul(out=rs02[:], in0=rs[:], scalar1=0.2)
                nmrs = tmp.tile([128, 1], F32, tag="nmrs")
                nc.vector.tensor_tensor(out=nmrs[:], in0=mv[:, 0:1], in1=rs08[:],
                                        op=mybir.AluOpType.mult)
                nc.vector.tensor_scalar_mul(out=nmrs[:], in0=nmrs[:], scalar1=-1.0)

                if out_pads is not None:
                    relu_t = tmp.tile([128, osz, osz], F16, tag="ev")
                    nc.scalar.activation(out=relu_t[:], in_=pm[:],
                                         func=mybir.ActivationFunctionType.Relu,
                                         bias=nmrs[:], scale=rs08[:])
                    lin_t = tmp.tile([128, osz, osz], F16, tag="ev")
                    nc.vector.tensor_scalar(out=lin_t[:], in0=pm[:],
                                            scalar1=mv[:, 0:1], scalar2=rs02[:],
                                            op0=mybir.AluOpType.subtract,
                                            op1=mybir.AluOpType.mult)
                    nc.vector.tensor_tensor(
                        out=out_pads[m][:, 1:1 + osz, 1:1 + osz],
                        in0=lin_t[:], in1=relu_t[:], op=mybir.AluOpType.add)
                else:
                    # pooled output only: materialize normalized lrelu then reduce
                    relu_t = tmp.tile([128, osz * osz], F32, tag="ev3")
                    nc.scalar.activation(out=relu_t[:], in_=pf,
                                         func=mybir.ActivationFunctionType.Relu,
                                         bias=nmrs[:], scale=rs08[:])
                    lin_t = tmp.tile([128, osz * osz], F32, tag="ev3")
                    nc.vector.tensor_scalar(out=lin_t[:], in0=pf,
                                            scalar1=mv[:, 0:1], scalar2=rs02[:],
                                            op0=mybir.AluOpType.subtract,
                                            op1=mybir.AluOpType.mult)
                    both = tmp.tile([128, osz * osz], F32, tag="ev3")
                    nc.vector.tensor_tensor(out=both[:], in0=lin_t[:], in1=relu_t[:],
                                            op=mybir.AluOpType.add)
                    nc.vector.tensor_reduce(out=pooled[:, m:m + 1], in_=both[:],
                                            axis=mybir.AxisListType.X,
                                            op=mybir.AluOpType.add)

        # conv1: 256 -> 512, 64x64 -> 32x32
        conv1_pad = [act.tile([128, S1 + 2, S1 + 2], F16, tag=f"c1p{m}", name=f"c1p{m}") for m in range(4)]
        with low_priority(tc, 800):
            for m in range(4):
                nc.gpsimd.memset(conv1_pad[m][:, 0, :], 0.0)
                nc.gpsimd.memset(conv1_pad[m][:, S1 + 1, :], 0.0)
                nc.gpsimd.memset(conv1_pad[m][:, :, 0], 0.0)
                nc.gpsimd.memset(conv1_pad[m][:, :, S1 + 1], 0.0)
        conv_norm(conv0_pad, d_w1, 4, 2, S1, conv1_pad)

        # conv2: 512 -> 1024, 32x32 -> 16x16
        conv2_pad = [act.tile([128, S2 + 2, S2 + 2], F16, tag=f"c2p{m}", name=f"c2p{m}") for m in range(8)]
        with low_priority(tc, 1200):
            for m in range(8):
                nc.gpsimd.memset(conv2_pad[m][:, 0, :], 0.0)
                nc.gpsimd.memset(conv2_pad[m][:, S2 + 1, :], 0.0)
                nc.gpsimd.memset(conv2_pad[m][:, :, 0], 0.0)
                nc.gpsimd.memset(conv2_pad[m][:, :, S2 + 1], 0.0)
        conv_norm(conv1_pad, d_w2, 8, 4, S2, conv2_pad)

        # conv3: 1024 -> 1024, 16x16 -> 8x8; only pooled means survive
        # Preload w3 chunks into recycled slots (sep_pad / conv0_pad / mask_im /
        # tsp are dead by now) plus the wch pool, to keep DMA busy mid-kernel.
        c3w = {}
        idx = 0
        specs = [(act, "sep_pad", 8), (act, "c0p0", 2), (act, "c0p1", 2),
                 (act, "mask_im", 2), (tsp, "tsp", 1), (tsp, "tsp", 1),
                 (tsp, "tsp", 1), (act, "c1p0", 1), (act, "c1p1", 1),
                 (act, "c1p2", 1), (act, "c1p3", 1)]
        for pool_, tag_, n_ in specs:
            t = pool_.tile([128, n_, 16, 128], F16, tag=tag_, name=f"w3pre{idx}")
            for j in range(n_):
                m_, k_ = divmod(idx, 8)
                nc.sync.dma_start(t[:, j], d_w3.ap()[m_, k_])
                c3w[(m_, k_)] = t[:, j]
                idx += 1
        for rest in range(idx, 64):
            m_, k_ = divmod(rest, 8)
            t = wch.tile([128, 16, 128], F16, tag="wch", name=f"w3c{rest}")
            nc.sync.dma_start(t[:], d_w3.ap()[m_, k_])
            c3w[(m_, k_)] = t[:]
        pooled32 = const.tile([128, 8], F32)
        conv_norm(conv2_pad, d_w3, 8, 8, S3, None, pooled=pooled32,
                  wk_provider=lambda m, k: c3w[(m, k)])

        # head: out = w4^T @ pooled (w4 pre-scaled by 1/64) + b4
        pooled16 = const.tile([128, 8], F16)
        nc.vector.tensor_copy(pooled16[:], pooled32[:])
        pH = ps.tile([128, 1], F32, tag="ps")
        for k in range(8):
            nc.tensor.matmul(pH[:], w4_sb[:, k, :], pooled16[:, k:k + 1],
                             start=(k == 0), stop=(k == 7))
        out_sb = const.tile([128, 1], F32)
        nc.vector.tensor_tensor(out=out_sb[:], in0=pH[:], in1=b4t[:],
                                op=mybir.AluOpType.add)
        nc.sync.dma_start(d_out.ap(), out_sb[:])

    nc.compile()
    return nc


_NC = None


def _get_nc():
    global _NC
    if _NC is None:
        _NC = _build_nc()
    return _NC


def _prep_shared(w0, b0, w1, w2, w3, w4, b4):
    f16 = np.float16
    idx = np.arange(W)
    ang = (2.0 * np.pi / W) * np.outer(idx, idx).astype(np.float32)
    twC = (np.cos(ang) / W).astype(np.float32)
    twS = (np.sin(ang) / W).astype(np.float32)
    twc2 = np.cos(ang).astype(f16)
    tws2 = np.sin(ang).astype(f16)

    # w0 sep part: [m, t, ci, co]
    w0f = np.asarray(w0, np.float32)
    w0s = np.empty((2, 16, 128, 128), f16)
    for m in range(2):
        for t in range(16):
            kh, kw = t // 4, t % 4
            w0s[m, t] = w0f[128 * m:128 * (m + 1), 0:128, kh, kw].T.astype(f16)
    # w0 mask part: rows (kh,kw,ci) matching host im2col order, cols (m,co)
    wm = np.zeros((96, 2, 128), f16)
    for kh in range(4):
        for kw in range(4):
            for m in range(2):
                wm[(kh * 4 + kw) * 6:(kh * 4 + kw) * 6 + 6, m, :] = \
                    w0f[128 * m:128 * (m + 1), 128:134, kh, kw].T.astype(f16)
    wm = wm.reshape(96, 256)

    def pack(wl, nm, nk):
        wlf = np.asarray(wl, np.float32)
        o = np.empty((nm, nk, 128, 16, 128), f16)
        for m in range(nm):
            for k in range(nk):
                for t in range(16):
                    kh, kw = t // 4, t % 4
                    o[m, k, :, t, :] = wlf[128 * m:128 * (m + 1),
                                           128 * k:128 * (k + 1), kh, kw].T.astype(f16)
        return o

    w1l = pack(w1, 4, 2)
    w2l = pack(w2, 8, 4)
    w3l = pack(w3, 8, 8)
    w4f = np.asarray(w4, np.float32)[:, :, 0, 0] / (S3 * S3)  # (128, 1024)
    w4l = np.empty((8, 128, 128), f16)
    for k in range(8):
        w4l[k] = w4f[:, 128 * k:128 * (k + 1)].T.astype(f16)

    b0f = np.asarray(b0, np.float32)
    b0t = np.stack([b0f[0:128], b0f[128:256], 0.8 * b0f[0:128], 0.8 * b0f[128:256]],
                   axis=1).astype(np.float32)  # (128, 4)
    b4t = np.asarray(b4, np.float32).reshape(128, 1)
    maskp_proto = None
    return dict(twC=twC, twS=twS, twc2=twc2, tws2=tws2, w0s=w0s, wm=wm,
                w1l=w1l, w2l=w2l, w3l=w3l, w4l=w4l, b0t=b0t, b4t=b4t)


def kernel(image, mask_embedding, w0, b0, w1, b1, w2, b2, w3, b3, w4, b4):
    from concourse.bass_utils import run_bass_kernel_spmd

    nc = _get_nc()
    shared = _prep_shared(w0, b0, w1, w2, w3, w4, b4)

    image = np.asarray(image, np.float32)
    mask = np.asarray(mask_embedding, np.float32)
    in_maps = []
    for b in range(B):
        mp = np.zeros((6, H + 2, W + 2), np.float16)
        mp[:, 1:H + 1, 1:W + 1] = mask[b].astype(np.float16)
        imcol = np.empty((96, S0, S0), np.float16)
        for kh in range(4):
            for kw in range(4):
                t = kh * 4 + kw
                imcol[t * 6:(t + 1) * 6] = mp[:, kh:kh + 2 * S0 - 1:2,
                                              kw:kw + 2 * S0 - 1:2]
        m = dict(shared)
        m["img"] = image[b, 0]
        m["maskim"] = imcol
        in_maps.append(m)

    res = run_bass_kernel_spmd(nc, in_maps, list(range(B)))
    out = np.stack([res.results[b]["out"] for b in range(B)]).astype(np.float32)
    return out



# revision 4
# speedup vs baseline: 1.4349x; 1.4349x over previous
"""Trainium2 Bass kernel for nn_EvaluatorNetwork.

Network (per sample):
  sep = per-column spectral decomposition of image  -> (128, 128, 128)
  x = concat([sep, mask_embedding]) -> (134, 128, 128)
  conv0 4x4 s2 (134->256) + b0, lrelu          -> (256, 64, 64)
  conv1 4x4 s2 (256->512), inorm, lrelu        -> (512, 32, 32)
  conv2 4x4 s2 (512->1024), inorm, lrelu       -> (1024, 16, 16)
  conv3 4x4 s2 (1024->1024), inorm, lrelu      -> (1024, 8, 8)
  avgpool -> (1024,); head 1024->128 + b4      -> (128,)

Sharding: pure data parallel, batch 8 over 8 NeuronCores; weights replicated.

Math notes:
  sep[i,h,w] = colRT[i,h]*cos(2pi*i*w/W) + colJT[i,h]*sin(2pi*i*w/W)
    colRT = C @ img^T, colJT = S @ img^T,  C/S[w',i] = cos/sin(2pi*i*w'/W)/W
  b1..b3 cancel exactly through instance norm (constant channel shift), so
  they are ignored; b0 and b4 are applied.
  lrelu(y) = 0.2*y + 0.8*relu(y); relu(0.8*y) = 0.8*relu(y) (used so the
  per-partition-scale Relu activation can produce the relu part directly).

fp8 strategy: all convs run as fp8e4m3 DoubleRow matmuls (2 fp8 MACs per PE
cell per cycle, K_eff=256).  Weight tensors are pre-scaled on the host so the
e4m3 range is well used; activations between layers are stored as fp8 scaled
by ACT_S.  Every conv except conv0 is followed by InstanceNorm, which is
scale-invariant, so all these scales cancel exactly; conv0's scale is divided
out during its PSUM evacuation (folded into the lrelu coefficients).
conv1..3 pair input-channel chunks (ci 2j,2j+1) per DoubleRow matmul; conv0
pairs adjacent kw taps (same kh) since its sep input has only one 128-channel
chunk.  The 6 mask channels go through a separate fp16 im2col matmul into the
same PSUM accumulation group (weights pre-scaled to match).
"""
from contextlib import ExitStack

import numpy as np
import ml_dtypes

import concourse.bass as bass
import concourse.tile as tile
from concourse import bacc, mybir
from concourse.masks import make_identity

F32 = mybir.dt.float32
F16 = mybir.dt.float16
F8 = mybir.dt.float8e4
E4 = ml_dtypes.float8_e4m3
DR = mybir.MatmulPerfMode.DoubleRow

B, H, W = 8, 128, 128
EPS = 1e-5
ACT_S = 16.0  # fp8 activation scale between layers
W_SCALE = 1024.0  # fp8 weight scale (init is ~N(0, 0.02^2); 1024*|w| << 240)
S0C = ACT_S * W_SCALE  # conv PSUM scale (fp8 act x fp8 weight)
# inorm on S0C-scaled PSUM: (Sx - Sm)/sqrt(S^2 v + S^2 eps) == true inorm
EPS_SCALED = EPS * S0C * S0C
_EV_RELU = 0.8 * ACT_S / S0C  # conv0 evac: relu-branch scale
_EV_LIN = 0.2 * ACT_S / S0C  # conv0 evac: linear-branch scale

# conv output spatial sizes
S0, S1, S2, S3 = 64, 32, 16, 8


def _build_nc():
    nc = bacc.Bacc("TRN2", target_bir_lowering=False, debug=False)

    # ---------------- DRAM parameters (per-core) ----------------
    d_img = nc.dram_tensor("img", [H, W], F32, kind="ExternalInput")
    d_maskim = nc.dram_tensor("maskim", [96, S0, S0], F16, kind="ExternalInput")
    d_C = nc.dram_tensor("twC", [W, W], F32, kind="ExternalInput")
    d_S = nc.dram_tensor("twS", [W, W], F32, kind="ExternalInput")
    d_c2 = nc.dram_tensor("twc2", [W, W], F16, kind="ExternalInput")
    d_s2 = nc.dram_tensor("tws2", [W, W], F16, kind="ExternalInput")
    # conv0 sep weights: [p=ci, m, u(tap pair), i(pair member), co] fp8
    d_w0s = nc.dram_tensor("w0s", [128, 2, 8, 2, 128], F8, kind="ExternalInput")
    d_wm = nc.dram_tensor("wm", [96, 256], F16, kind="ExternalInput")
    # convl weights: [m, j(ci-chunk pair), p=ci_lo, i(pair member), t, co] fp8
    d_w1 = nc.dram_tensor("w1l", [4, 1, 128, 2, 16, 128], F8, kind="ExternalInput")
    d_w2 = nc.dram_tensor("w2l", [8, 2, 128, 2, 16, 128], F8, kind="ExternalInput")
    d_w3 = nc.dram_tensor("w3l", [8, 4, 128, 2, 16, 128], F8, kind="ExternalInput")
    d_w4 = nc.dram_tensor("w4l", [8, 128, 128], F16, kind="ExternalInput")
    d_b0 = nc.dram_tensor("b0t", [128, 4], F32, kind="ExternalInput")
    d_b4 = nc.dram_tensor("b4t", [128, 1], F32, kind="ExternalInput")
    d_out = nc.dram_tensor("out", [128], F32, kind="ExternalOutput")

    from contextlib import contextmanager

    @contextmanager
    def low_priority(tc, bump):
        orig = tc.cur_priority
        tc.cur_priority = orig + bump
        try:
            yield
        finally:
            tc.cur_priority = orig

    with tile.TileContext(nc) as tc, ExitStack() as ctx:
        const = ctx.enter_context(tc.tile_pool(name="const", bufs=1))
        act = ctx.enter_context(tc.tile_pool(name="act", bufs=1))
        wch = ctx.enter_context(tc.tile_pool(name="wch", bufs=26))
        ps = ctx.enter_context(tc.tile_pool(name="ps", bufs=3, space="PSUM"))
        tmp = ctx.enter_context(tc.tile_pool(name="tmp", bufs=6))
        tsp = ctx.enter_context(tc.tile_pool(name="tsp", bufs=3))

        # ---------------- constants / inputs ----------------
        ident = const.tile([128, 128], F32)
        make_identity(nc, ident[:])
        img32 = const.tile([128, 128], F32)
        nc.sync.dma_start(img32[:], d_img.ap())
        twC = const.tile([128, 128], F32)
        nc.sync.dma_start(twC[:], d_C.ap())
        twS = const.tile([128, 128], F32)
        nc.sync.dma_start(twS[:], d_S.ap())
        c2ated = const.tile([128, 128], F16)
        nc.sync.dma_start(c2ated[:], d_c2.ap())
        s2ated = const.tile([128, 128], F16)
        nc.sync.dma_start(s2ated[:], d_s2.ap())
        b0t = const.tile([128, 4], F32)
        nc.sync.dma_start(b0t[:], d_b0.ap())
        b4t = const.tile([128, 1], F32)
        nc.sync.dma_start(b4t[:], d_b4.ap())

        # mask im2col (host-computed): rows (kh,kw,ci), cols (oh,ow)
        mask_im = act.tile([96, S0, S0], F16)
        nc.sync.dma_start(mask_im[:], d_maskim.ap())

        # small weights resident
        w0t = const.tile([128, 2, 8, 2, 128], F8)
        nc.sync.dma_start(w0t[:], d_w0s.ap())
        wm_sb = const.tile([96, 256], F16)
        nc.sync.dma_start(wm_sb[:], d_wm.ap())
        w4_sb = const.tile([128, 8, 128], F16)
        nc.sync.dma_start(w4_sb[:], d_w4.ap().rearrange("t k c -> k t c"))
        w1t = [const.tile([128, 2, 16, 128], F8, name=f"w1t{m}") for m in range(4)]
        for m in range(4):
            nc.sync.dma_start(w1t[m][:], d_w1.ap()[m, 0])

        # w2 (16 pair-tiles) + w3 (32 pair-tiles) stream through wch
        wstream = {}
        for m in range(8):
            for j in range(2):
                t_ = wch.tile([128, 2, 16, 128], F8, tag="wch", name=f"w2c{m}_{j}")
                nc.sync.dma_start(t_[:], d_w2.ap()[m, j])
                wstream[(2, m, j)] = t_
        for m in range(8):
            for j in range(4):
                t_ = wch.tile([128, 2, 16, 128], F8, tag="wch", name=f"w3c{m}_{j}")
                nc.sync.dma_start(t_[:], d_w3.ap()[m, j])
                wstream[(3, m, j)] = t_

        # ---------------- spectral map ----------------
        pT = ps.tile([128, 128], F32, tag="ps")
        nc.tensor.transpose(pT[:], img32[:], ident[:])
        imgT32 = const.tile([128, 128], F32)
        nc.vector.tensor_copy(imgT32[:], pT[:])

        pR = ps.tile([128, 128], F32, tag="ps")
        nc.tensor.matmul(pR[:], twC[:], imgT32[:], start=True, stop=True)
        colRT = const.tile([128, 128], F16)
        nc.vector.tensor_copy(colRT[:], pR[:])
        pJ = ps.tile([128, 128], F32, tag="ps")
        nc.tensor.matmul(pJ[:], twS[:], imgT32[:], start=True, stop=True)
        colJT = const.tile([128, 128], F16)
        nc.vector.tensor_copy(colJT[:], pJ[:])

        # sep_pad holds ACT_S * sep (the ACT_S is folded into twC/twS on host)
        sep_pad = act.tile([128, H + 2, W + 2], F8)
        nc.gpsimd.memset(sep_pad[:, 0, :], 0.0)
        nc.gpsimd.memset(sep_pad[:, H + 1, :], 0.0)
        nc.gpsimd.memset(sep_pad[:, :, 0], 0.0)
        nc.gpsimd.memset(sep_pad[:, :, W + 1], 0.0)

        h_chunks = [4, 4, 8] + [16] * 7  # small first chunks: conv0 starts sooner
        h0 = 0
        for HC in h_chunks:
            # A-term: colRT[i,h] bcast over w;  B-term: c2[i,w] bcast over h
            cR = colRT[:, h0:h0 + HC]
            aR = bass.AP(tensor=cR.tensor, offset=cR.offset,
                         ap=[cR.ap[0], [1, HC], [0, W]])
            cJ = colJT[:, h0:h0 + HC]
            aJ = bass.AP(tensor=cJ.tensor, offset=cJ.offset,
                         ap=[cJ.ap[0], [1, HC], [0, W]])
            c2a = c2ated[:, :]
            b2 = bass.AP(tensor=c2a.tensor, offset=c2a.offset,
                         ap=[c2a.ap[0], [0, HC], [1, W]])
            s2a = s2ated[:, :]
            b3 = bass.AP(tensor=s2a.tensor, offset=s2a.offset,
                         ap=[s2a.ap[0], [0, HC], [1, W]])
            t1 = tsp.tile([128, 16, W], F16, tag="tsp", name="t1")[:, :HC, :]
            nc.gpsimd.tensor_tensor(out=t1[:], in0=aR, in1=b2, op=mybir.AluOpType.mult)
            t2 = tsp.tile([128, 16, W], F16, tag="tsp", name="t2")[:, :HC, :]
            nc.vector.tensor_tensor(out=t2[:], in0=aJ, in1=b3, op=mybir.AluOpType.mult)
            nc.vector.tensor_tensor(out=sep_pad[:, 1 + h0:1 + h0 + HC, 1:1 + W],
                                    in0=t1[:], in1=t2[:], op=mybir.AluOpType.add)
            h0 += HC

        # ---------------- conv0: 134 -> 256, 128x128 -> 64x64 ----------------
        # sep part: fp8 DoubleRow over tap pairs (kw even/odd, same kh).
        # mask part: fp16 im2col matmul into the same PSUM group.
        c0pad = act.tile([128, 2, S0 + 2, S0 + 2], F8)
        with low_priority(tc, 400):
            for m in range(2):
                nc.gpsimd.memset(c0pad[:, m, 0, :], 0.0)
                nc.gpsimd.memset(c0pad[:, m, S0 + 1, :], 0.0)
                nc.gpsimd.memset(c0pad[:, m, :, 0], 0.0)
                nc.gpsimd.memset(c0pad[:, m, :, S0 + 1], 0.0)

        row = W + 2  # sep_pad row stride (elements)
        OHB0 = 8  # oh rows per chunk -> N = 8*64 = 512
        for m in range(2):
            for ch in range(S0 // OHB0):
                oh0 = ch * OHB0
                p0 = ps.tile([128, OHB0, S0], F32, tag="ps")
                for u in range(8):
                    kh, kw0 = (2 * u) // 4, (2 * u) % 4
                    base = sep_pad[:, kh + 2 * oh0, kw0]
                    rhs = bass.AP(tensor=base.tensor, offset=base.offset,
                                  ap=[base.ap[0], [1, 2], [2 * row, OHB0], [2, S0]])
                    nc.tensor.matmul(p0[:], w0t[:, m, u, :, :], rhs,
                                     start=(u == 0), stop=False, perf_mode=DR)
                nc.tensor.matmul(p0[:], wm_sb[:, m * 128:(m + 1) * 128],
                                 mask_im[:, oh0:oh0 + OHB0, :],
                                 start=False, stop=True)
                # evac: ACT_S*lrelu(x/S0c + b0) = lin + relu parts (coeffs hosted)
                relu_t = tmp.tile([128, OHB0, S0], F16, tag="ev")
                nc.scalar.activation(out=relu_t[:], in_=p0[:],
                                     func=mybir.ActivationFunctionType.Relu,
                                     bias=b0t[:, 2 + m:3 + m], scale=float(_EV_RELU))
                lin_t = tmp.tile([128, OHB0, S0], F16, tag="ev")
                nc.vector.tensor_scalar(out=lin_t[:], in0=p0[:],
                                        scalar1=b0t[:, m:m + 1], scalar2=float(_EV_LIN),
                                        op0=mybir.AluOpType.add,
                                        op1=mybir.AluOpType.mult)
                nc.vector.tensor_tensor(
                    out=c0pad[:, m, 1 + oh0:1 + oh0 + OHB0, 1:1 + S0],
                    in0=lin_t[:], in1=relu_t[:], op=mybir.AluOpType.add)

        # ---------------- generic fp8 strided conv layer with inorm ----------
        def conv_norm(x_pad, nk, layer, nm, osz, out_pad=None, pooled=None,
                      wk_provider=None):
            """x_pad: tile (128, nk, isz+2, isz+2) fp8, chunks along dim1.
            out_pad: tile (128, nm, osz+2, osz+2) fp8 (scaled by ACT_S), or
            None with pooled (128, nm) fp32."""
            n_spatial = osz * osz
            ohb = max(1, min(osz, 512 // osz))
            nch = osz // ohb
            for m in range(nm):
                pm = ps.tile([128, osz, osz], F32, tag="ps")
                for ch in range(nch):
                    oh0 = ch * ohb
                    pslice = pm[:, oh0:oh0 + ohb, :]
                    first = True
                    for j in range(nk // 2):
                        wt = wk_provider(layer, m, j)
                        for t in range(16):
                            kh, kw = t // 4, t % 4
                            rhs = x_pad[:, 2 * j:2 * j + 2,
                                        kh + 2 * oh0: kh + 2 * oh0 + 2 * ohb - 1: 2,
                                        kw: kw + 2 * osz - 1: 2]
                            nc.tensor.matmul(pslice, wt[:, :, t, :], rhs,
                                             start=first,
                                             stop=(j == nk // 2 - 1 and t == 15),
                                             perf_mode=DR)
                            first = False
                # instance norm stats over full spatial
                nsub = max(1, n_spatial // 512)
                sub = n_spatial // nsub
                stats = tmp.tile([128, nsub, 6], F32, tag="st")
                pf = pm[:].rearrange("p a b -> p (a b)")
                for s in range(nsub):
                    nc.vector.bn_stats(out=stats[:, s, :], in_=pf[:, s * sub:(s + 1) * sub])
                mv = tmp.tile([128, 2], F32, tag="mv")
                nc.vector.bn_aggr(out=mv[:], in_=stats[:])
                eps_t = tmp.tile([128, 1], F32, tag="eps")
                nc.vector.memset(eps_t[:], EPS_SCALED)
                rs = tmp.tile([128, 1], F32, tag="rs")
                nc.scalar.activation(out=rs[:], in_=mv[:, 1:2],
                                     func=mybir.ActivationFunctionType.Sqrt,
                                     bias=eps_t[:], scale=1.0)
                nc.vector.reciprocal(out=rs[:], in_=rs[:])
                out_s = ACT_S if out_pad is not None else 1.0
                rs08 = tmp.tile([128, 1], F32, tag="rs08")
                nc.vector.tensor_scalar_mul(out=rs08[:], in0=rs[:], scalar1=0.8 * out_s)
                rs02 = tmp.tile([128, 1], F32, tag="rs02")
                nc.vector.tensor_scalar_mul(out=rs02[:], in0=rs[:], scalar1=0.2 * out_s)
                nmrs = tmp.tile([128, 1], F32, tag="nmrs")
                nc.vector.tensor_tensor(out=nmrs[:], in0=mv[:, 0:1], in1=rs08[:],
                                        op=mybir.AluOpType.mult)
                nc.vector.tensor_scalar_mul(out=nmrs[:], in0=nmrs[:], scalar1=-1.0)

                if out_pad is not None:
                    relu_t = tmp.tile([128, osz, osz], F16, tag="ev")
                    nc.scalar.activation(out=relu_t[:], in_=pm[:],
                                         func=mybir.ActivationFunctionType.Relu,
                                         bias=nmrs[:], scale=rs08[:])
                    lin_t = tmp.tile([128, osz, osz], F16, tag="ev")
                    nc.vector.tensor_scalar(out=lin_t[:], in0=pm[:],
                                            scalar1=mv[:, 0:1], scalar2=rs02[:],
                                            op0=mybir.AluOpType.subtract,
                                            op1=mybir.AluOpType.mult)
                    nc.vector.tensor_tensor(
                        out=out_pad[:, m, 1:1 + osz, 1:1 + osz],
                        in0=lin_t[:], in1=relu_t[:], op=mybir.AluOpType.add)
                else:
                    # pooled output only: materialize normalized lrelu then reduce
                    relu_t = tmp.tile([128, osz * osz], F32, tag="ev3")
                    nc.scalar.activation(out=relu_t[:], in_=pf,
                                         func=mybir.ActivationFunctionType.Relu,
                                         bias=nmrs[:], scale=rs08[:])
                    lin_t = tmp.tile([128, osz * osz], F32, tag="ev3")
                    nc.vector.tensor_scalar(out=lin_t[:], in0=pf,
                                            scalar1=mv[:, 0:1], scalar2=rs02[:],
                                            op0=mybir.AluOpType.subtract,
                                            op1=mybir.AluOpType.mult)
                    both = tmp.tile([128, osz * osz], F32, tag="ev3")
                    nc.vector.tensor_tensor(out=both[:], in0=lin_t[:], in1=relu_t[:],
                                            op=mybir.AluOpType.add)
                    nc.vector.tensor_reduce(out=pooled[:, m:m + 1], in_=both[:],
                                            axis=mybir.AxisListType.X,
                                            op=mybir.AluOpType.add)

        def w_at(layer, m, j):
            if layer == 1:
                return w1t[m]
            return wstream[(layer, m, j)]

        # conv1: 256 -> 512, 64x64 -> 32x32
        c1pad = act.tile([128, 4, S1 + 2, S1 + 2], F8)
        with low_priority(tc, 800):
            for m in range(4):
                nc.gpsimd.memset(c1pad[:, m, 0, :], 0.0)
                nc.gpsimd.memset(c1pad[:, m, S1 + 1, :], 0.0)
                nc.gpsimd.memset(c1pad[:, m, :, 0], 0.0)
                nc.gpsimd.memset(c1pad[:, m, :, S1 + 1], 0.0)
        conv_norm(c0pad, 2, 1, 4, S1, out_pad=c1pad, wk_provider=w_at)

        # conv2: 512 -> 1024, 32x32 -> 16x16
        c2pad = act.tile([128, 8, S2 + 2, S2 + 2], F8)
        with low_priority(tc, 1200):
            for m in range(8):
                nc.gpsimd.memset(c2pad[:, m, 0, :], 0.0)
                nc.gpsimd.memset(c2pad[:, m, S2 + 1, :], 0.0)
                nc.gpsimd.memset(c2pad[:, m, :, 0], 0.0)
                nc.gpsimd.memset(c2pad[:, m, :, S2 + 1], 0.0)
        conv_norm(c1pad, 4, 2, 8, S2, out_pad=c2pad, wk_provider=w_at)

        # conv3: 1024 -> 1024, 16x16 -> 8x8; only pooled means survive
        pooled32 = const.tile([128, 8], F32)
        conv_norm(c2pad, 8, 3, 8, S3, pooled=pooled32, wk_provider=w_at)

        # head: out = w4^T @ pooled (w4 pre-scaled by 1/64) + b4
        pooled16 = const.tile([128, 8], F16)
        nc.vector.tensor_copy(pooled16[:], pooled32[:])
        pH = ps.tile([128, 1], F32, tag="ps")
        for k in range(8):
            nc.tensor.matmul(pH[:], w4_sb[:, k, :], pooled16[:, k:k + 1],
                             start=(k == 0), stop=(k == 7))
        out_sb = const.tile([128, 1], F32)
        nc.vector.tensor_tensor(out=out_sb[:], in0=pH[:], in1=b4t[:],
                                op=mybir.AluOpType.add)
        nc.sync.dma_start(d_out.ap(), out_sb[:])

    nc.compile()
    return nc


_NC = None


def _get_nc():
    global _NC
    if _NC is None:
        _NC = _build_nc()
    return _NC


def _q8(x, scale):
    return np.clip(np.asarray(x, np.float32) * scale, -240.0, 240.0).astype(E4)


def _prep_shared(w0, b0, w1, w2, w3, w4, b4):
    f16 = np.float16
    idx = np.arange(W)
    ang = (2.0 * np.pi / W) * np.outer(idx, idx).astype(np.float32)
    # ACT_S folded into the column transforms so sep_pad = ACT_S * sep
    twC = (ACT_S * np.cos(ang) / W).astype(np.float32)
    twS = (ACT_S * np.sin(ang) / W).astype(np.float32)
    twc2 = np.cos(ang).astype(f16)
    tws2 = np.sin(ang).astype(f16)

    w0f = np.asarray(w0, np.float32)
    s_w0 = W_SCALE
    S0c = S0C

    # w0 sep part: [p=ci, m, u, i, co] fp8, taps (2u, 2u+1) = (kh, kw0/kw0+1)
    w0s = np.empty((128, 2, 8, 2, 128), E4)
    for m in range(2):
        for u in range(8):
            kh, kw0 = (2 * u) // 4, (2 * u) % 4
            for i in range(2):
                w0s[:, m, u, i, :] = _q8(
                    w0f[128 * m:128 * (m + 1), 0:128, kh, kw0 + i].T, s_w0)
    # w0 mask part: rows (kh,kw,ci) matching host im2col order, cols (m,co),
    # pre-scaled by S0c to match the fp8 sep-part PSUM scale.
    wm = np.zeros((96, 2, 128), f16)
    for kh in range(4):
        for kw in range(4):
            for m in range(2):
                wm[(kh * 4 + kw) * 6:(kh * 4 + kw) * 6 + 6, m, :] = \
                    (S0c * w0f[128 * m:128 * (m + 1), 128:134, kh, kw].T).astype(f16)
    wm = wm.reshape(96, 256)

    def pack8(wl, nm, nk):
        wlf = np.asarray(wl, np.float32)
        s = W_SCALE
        o = np.empty((nm, nk // 2, 128, 2, 16, 128), E4)
        for m in range(nm):
            for j in range(nk // 2):
                for i in range(2):
                    for t in range(16):
                        kh, kw = t // 4, t % 4
                        o[m, j, :, i, t, :] = _q8(
                            wlf[128 * m:128 * (m + 1),
                                128 * (2 * j + i):128 * (2 * j + i + 1),
                                kh, kw].T, s)
        return o

    w1l = pack8(w1, 4, 2)
    w2l = pack8(w2, 8, 4)
    w3l = pack8(w3, 8, 8)
    w4f = np.asarray(w4, np.float32)[:, :, 0, 0] / (S3 * S3)  # (128, 1024)
    w4l = np.empty((8, 128, 128), f16)
    for k in range(8):
        w4l[k] = w4f[:, 128 * k:128 * (k + 1)].T.astype(f16)

    b0f = np.asarray(b0, np.float32)
    # cols: [S0c*b0 (m=0), S0c*b0 (m=1), 0.8*ACT_S*b0 (m=0), 0.8*ACT_S*b0 (m=1)]
    b0t = np.stack([S0c * b0f[0:128], S0c * b0f[128:256],
                    0.8 * ACT_S * b0f[0:128], 0.8 * ACT_S * b0f[128:256]],
                   axis=1).astype(np.float32)
    b4t = np.asarray(b4, np.float32).reshape(128, 1)
    return dict(twC=twC, twS=twS, twc2=twc2, tws2=tws2, w0s=w0s, wm=wm,
                w1l=w1l, w2l=w2l, w3l=w3l, w4l=w4l, b0t=b0t, b4t=b4t)


def kernel(image, mask_embedding, w0, b0, w1, b1, w2, b2, w3, b3, w4, b4):
    from concourse.bass_utils import run_bass_kernel_spmd

    nc = _get_nc()
    shared = _prep_shared(w0, b0, w1, w2, w3, w4, b4)

    image = np.asarray(image, np.float32)
    mask = np.asarray(mask_embedding, np.float32)
    in_maps = []
    for b in range(B):
        mp = np.zeros((6, H + 2, W + 2), np.float16)
        mp[:, 1:H + 1, 1:W + 1] = mask[b].astype(np.float16)
        imcol = np.empty((96, S0, S0), np.float16)
        for kh in range(4):
            for kw in range(4):
                t = kh * 4 + kw
                imcol[t * 6:(t + 1) * 6] = mp[:, kh:kh + 2 * S0 - 1:2,
                                              kw:kw + 2 * S0 - 1:2]
        m = dict(shared)
        m["img"] = image[b, 0]
        m["maskim"] = imcol
        in_maps.append(m)

    res = run_bass_kernel_spmd(nc, in_maps, list(range(B)))
    out = np.stack([res.results[b]["out"] for b in range(B)]).astype(np.float32)
    return out
